# revision 10
# baseline (speedup 1.0000x reference)
"""MultiHeadAttention (CLUSTERING softmax over query axis) on 8 Trainium2 cores.

Sharding: batch B=8, one batch element per NeuronCore (pure data parallel,
no collectives).

Per-core computation (L=1024, D=1024, H=16, HD=64):
  QT = (x_q @ Wq)^T            [d, l]   (bq dropped: cancels in softmax over q)
  KT = (x_k @ Wk + bk)^T       [d, l]
  V  = x_v @ Wv + bv           [l, d]
  per head h: ST_h[k, q] = QT_h . KT_h  (contraction over hd=64)
  E = exp(ST / 32)  with fused row-sums over q (free axis)
  r = 1/sums; V'_h[k, :] = V_h[k, :] * r_h[k]   (normalizer folded into V)
  OT_h[d, q] = sum_k V'_h[k, d] * E_h[k, q]
  y = OT^T @ Wo + bo           [l, d]

v3 structure (HW-calibrated):
  - Every K=128 contraction chunk is split into two K=64 matmuls on
    row-disjoint halves of the PE array, accumulating into two separate
    PSUM banks (combined by one DVE add). Row-alternating half matmuls
    let LDWEIGHTS overlap in-flight matmuls: measured 94ns/pair vs
    388ns for a full-K matmul on HW.
  - All biases applied via rank-1 matmuls into the A-half bank.
  - All inputs bf16 from host; ScalarE does exp only; y bf16 out.
"""

import math
from contextlib import ExitStack, nullcontext

import numpy as np

import concourse.bass as bass
import concourse.tile as tile
from concourse import mybir
from concourse.bass import ts

F32 = mybir.dt.float32
BF16 = mybir.dt.bfloat16
EXP = mybir.ActivationFunctionType.Exp
COPY = mybir.ActivationFunctionType.Copy
ADD = mybir.AluOpType.add

L = 1024
D = 1024
P = 128
NT = 8  # 1024 / 128
N_CORES = 8
SCALE = 1.0 / math.sqrt(D)


# ---------------------------------------------------------------------------
# Workaround: this walrus build supports very few sync-wait commands per
# instruction. Tile's kernel-tail drain / barriers can carry more. Move
# excess waits onto same-engine NOPs inserted immediately before (engines
# execute their stream in order, so this preserves semantics).
def split_excess_waits(nc):
    f = nc.m.functions[0]
    ctr = 0
    for b in f.blocks:
        insts = b.instructions
        i = 0
        while i < len(insts):
            inst = insts[i]
            si = inst.sync_info
            limit = 0 if "Drain" in type(inst).__name__ else 1
            if si is not None and si.on_wait and len(si.on_wait) > limit:
                waits = list(si.on_wait)
                keep = waits[-limit:] if limit else []
                extra = waits[: len(waits) - limit]
                pos = i
                for j in range(0, len(extra), 1):
                    nop = mybir.InstNoOp(name=f"waitsplit-{ctr}", ins=[], outs=[])
                    ctr += 1
                    nop.engine = inst.engine
                    nop.bass_nofuse = True
                    nop.sync_info = mybir.SyncInfo(
                        on_wait=[extra[j]], on_update=[]
                    )
                    insts.insert(pos, nop)
                    pos += 1
                    i += 1
                inst.sync_info = mybir.SyncInfo(
                    on_wait=keep, on_update=list(si.on_update)
                )
            i += 1


# ---------------------------------------------------------------------------
def _emit_body(nc, tc, ctx, t):
    persist = ctx.enter_context(tc.tile_pool(name="persist", bufs=1))
    pairp = ctx.enter_context(tc.tile_pool(name="pairp", bufs=2, space="PSUM"))

    # ---- constants -------------------------------------------------------
    ones_t = persist.tile([1, 512], BF16, name="ones")
    nc.vector.memset(ones_t[:], 1.0)
    bkb = persist.tile([1, D], BF16, name="bkb")
    nc.sync.dma_start(bkb[0:1, :], t["bkb"][None, :])
    bo_bf = persist.tile([1, D], BF16, name="bo")
    nc.sync.dma_start(bo_bf[0:1, :], t["bo"][None, :])
    bv_bf = persist.tile([1, D], BF16, name="bv")
    nc.sync.dma_start(bv_bf[0:1, :], t["bv"][None, :])
    ident = persist.tile([P, P], BF16, name="ident")
    nc.sync.dma_start(ident[:], t["ident"][:, :])

    # ---- persistent SBUF tensors ----------------------------------------
    xqT = [persist.tile([P, D], BF16, name=f"xqT{i}") for i in range(NT)]
    xkT = [persist.tile([P, D], BF16, name=f"xkT{i}") for i in range(NT)]
    wo_bf = [persist.tile([P, D], BF16, name=f"wo{i}") for i in range(NT)]
    v_sb = [persist.tile([P, D], BF16, name=f"v{i}") for i in range(NT)]
    ot_sb = [persist.tile([P, D], BF16, name=f"ot{i}") for i in range(NT)]
    ypart = [persist.tile([P, D], BF16, name=f"yp{i}") for i in range(NT)]
    tmpp = ctx.enter_context(tc.tile_pool(name="tmpp", bufs=4))

    # combine split-K banks: psB -> SBUF staging copy (on `cp_eng`), then
    # one legal PSUM+SBUF add on DVE.
    def combine(out_ap, psA, psB, cp_eng):
        tmp = tmpp.tile([P, 512], F32, name="tmb")
        if cp_eng == "act":
            nc.scalar.activation(tmp[:], psB[:], COPY)
        else:
            nc.vector.tensor_copy(tmp[:], psB[:])
        nc.vector.tensor_tensor(out_ap, psA[:], tmp[:], ADD)

    # Split-K pair chain: each 128-row term contributes rows 0:64 to bank A
    # and rows 64:128 to bank B, alternating so LDWEIGHTS overlaps the
    # in-flight matmul on the other row half. biasA: rank-1 (lhsT_row,
    # rhs_row) matmul appended to the bank-A group.
    def pair_chain(psA, psB, terms, biasA=None):
        n = len(terms)
        for i, (lhsT, rhs) in enumerate(terms):
            nc.tensor.matmul(
                psA[:], lhsT[0:64, :], rhs[0:64, :],
                start=(i == 0),
                stop=(i == n - 1 and biasA is None),
            )
            nc.tensor.matmul(
                psB[:], lhsT[64:128, :], rhs[64:128, :],
                start=(i == 0), stop=(i == n - 1),
            )
        if biasA is not None:
            blh, brh = biasA
            nc.tensor.matmul(psA[:], blh, brh, start=False, stop=True)

    # ---- front: x transposes + V projection ------------------------------
    with ExitStack() as front:
        xstage = front.enter_context(tc.tile_pool(name="xstage", bufs=5))
        xvtp = front.enter_context(tc.tile_pool(name="xvtp", bufs=1))
        wvp = front.enter_context(tc.tile_pool(name="wvp", bufs=1))
        tpp = front.enter_context(tc.tile_pool(name="tpp", bufs=2, space="PSUM"))

        xvT = [xvtp.tile([P, D], BF16, name=f"xvT{i}") for i in range(NT)]
        wv_bf = [wvp.tile([P, D], BF16, name=f"wv{i}") for i in range(NT)]

        def xpath(xT_tiles, xdram):
            for half in range(2):
                xts = []
                for lt in range(4 * half, 4 * half + 4):
                    xs = xstage.tile([P, D], BF16, name="xs")
                    nc.sync.dma_start(xs[:], xdram[ts(lt, P), :])
                    xts.append(xs)
                for ct in range(NT):
                    tp = tpp.tile([P, 512], BF16, name="tp")
                    for i in range(4):
                        nc.tensor.transpose(
                            tp[:, ts(i, P)], xts[i][:, ts(ct, P)], ident[:]
                        )
                    nc.vector.tensor_copy(
                        xT_tiles[ct][:, 512 * half : 512 * half + 512], tp[:]
                    )

        xpath(xqT, t["xq"])
        xpath(xkT, t["xk"])
        xpath(xvT, t["xv"])
        for i in range(NT):
            nc.sync.dma_start(wv_bf[i][:], t["wv"][ts(i, P), :])

        # V[l, d] = x_v @ Wv + bv  (split-K pair chains)
        for lt in range(NT):
            for dc in range(2):
                psA = pairp.tile([P, 512], F32, name="pA")
                psB = pairp.tile([P, 512], F32, name="pB")
                terms = [
                    (xvT[ct][:, ts(lt, P)], wv_bf[ct][:, ts(dc, 512)])
                    for ct in range(NT)
                ]
                pair_chain(
                    psA, psB, terms,
                    biasA=(ones_t[0:1, 0:P], bv_bf[0:1, ts(dc, 512)]),
                )
                combine(v_sb[lt][:, ts(dc, 512)], psA, psB, "act")

        for i in range(NT):
            nc.sync.dma_start(wo_bf[i][:], t["wo"][ts(i, P), :])

    # ---- main attention loop ---------------------------------------------
    wqk = ctx.enter_context(tc.tile_pool(name="wqk", bufs=2))
    qtkt = ctx.enter_context(tc.tile_pool(name="qtkt", bufs=2))
    epool = ctx.enter_context(tc.tile_pool(name="epool", bufs=4))
    sums = ctx.enter_context(tc.tile_pool(name="sums", bufs=4))
    vppool = ctx.enter_context(tc.tile_pool(name="vppool", bufs=2))
    stq = ctx.enter_context(tc.tile_pool(name="stq", bufs=2, space="PSUM"))

    def emit_proj(hp, wdram, xT, out_tag):
        w_t = wqk.tile([P, D], BF16, name=f"w{out_tag}")
        nc.sync.dma_start(w_t[:], wdram[ts(hp, P), :])
        out_t = qtkt.tile([P, L], BF16, name=out_tag)
        for lc in range(2):
            psA = pairp.tile([P, 512], F32, name="pA")
            psB = pairp.tile([P, 512], F32, name="pB")
            terms = [
                (w_t[:, ts(ct, P)], xT[ct][:, ts(lc, 512)])
                for ct in range(NT)
            ]
            bias = None
            if out_tag == "kt":
                bias = (bkb[0:1, ts(hp, P)], ones_t[0:1, 0:512])
            pair_chain(psA, psB, terms, biasA=bias)
            combine(out_t[:, ts(lc, 512)], psA, psB, "dve")
        return out_t

    def emit_av(prev):
        # One PSUM bank per open accumulation group (start_tensor_calc
        # zeroes the whole bank, so A/B halves must not share).
        hp, e0, e1, vp = prev
        for qc in range(2):
            avA = pairp.tile([P, 512], F32, name="pA")
            avB = pairp.tile([P, 512], F32, name="pB")
            for kt in range(NT):
                nc.tensor.matmul(
                    avA[0:64, :],
                    vp[:, kt, 0:64],
                    e0[:, kt, ts(qc, 512)],
                    start=(kt == 0),
                    stop=(kt == NT - 1),
                )
                nc.tensor.matmul(
                    avB[64:128, :],
                    vp[:, kt, 64:128],
                    e1[:, kt, ts(qc, 512)],
                    start=(kt == 0),
                    stop=(kt == NT - 1),
                )
            nc.vector.tensor_copy(ot_sb[hp][0:64, ts(qc, 512)], avA[0:64, :])
            nc.vector.tensor_copy(
                ot_sb[hp][64:128, ts(qc, 512)], avB[64:128, :]
            )

    def emit_scores(hp, qt, kt_t):
        e0 = epool.tile([P, NT, L], BF16, name="e")
        e1 = epool.tile([P, NT, L], BF16, name="e")
        s0 = sums.tile([P, NT], F32, name="esum")
        s1 = sums.tile([P, NT], F32, name="esum")
        for kt in range(NT):
            st0 = stq.tile([P, L], F32, name="st")
            st1 = stq.tile([P, L], F32, name="st")
            for qc in range(2):
                nc.tensor.matmul(
                    st0[:, ts(qc, 512)],
                    kt_t[0:64, ts(kt, P)],
                    qt[0:64, ts(qc, 512)],
                    start=True,
                    stop=True,
                )
                nc.tensor.matmul(
                    st1[:, ts(qc, 512)],
                    kt_t[64:128, ts(kt, P)],
                    qt[64:128, ts(qc, 512)],
                    start=True,
                    stop=True,
                )
            nc.scalar.activation(
                e0[:, kt, :], st0[:], EXP, scale=SCALE,
                accum_out=s0[:, kt : kt + 1],
            )
            nc.scalar.activation(
                e1[:, kt, :], st1[:], EXP, scale=SCALE,
                accum_out=s1[:, kt : kt + 1],
            )
        r0 = sums.tile([P, NT], F32, name="r")
        r1 = sums.tile([P, NT], F32, name="r")
        nc.vector.reciprocal(r0[:], s0[:])
        nc.vector.reciprocal(r1[:], s1[:])
        vp = vppool.tile([P, NT, P], BF16, name="vp")
        for kt in range(NT):
            nc.vector.tensor_scalar_mul(
                vp[:, kt, 0:64],
                v_sb[kt][:, hp * P : hp * P + 64],
                r0[:, kt : kt + 1],
            )
            nc.vector.tensor_scalar_mul(
                vp[:, kt, 64:128],
                v_sb[kt][:, hp * P + 64 : hp * P + 128],
                r1[:, kt : kt + 1],
            )
        return (hp, e0, e1, vp)

    def outproj_batch1():
        # contract pairs 0..6 into y partials while pair 7 is in flight
        for lt in range(NT):
            for nc2 in range(2):
                psA = pairp.tile([P, 512], F32, name="pA")
                psB = pairp.tile([P, 512], F32, name="pB")
                terms = [
                    (ot_sb[dt][:, ts(lt, P)], wo_bf[dt][:, ts(nc2, 512)])
                    for dt in range(NT - 1)
                ]
                pair_chain(psA, psB, terms)
                combine(ypart[lt][:, ts(nc2, 512)], psA, psB, "act")

    def outproj_batch2():
        for lt in range(NT):
            yt = qtkt.tile([P, L], BF16, name="qt")
            for nc2 in range(2):
                psA = pairp.tile([P, 512], F32, name="pA")
                psB = pairp.tile([P, 512], F32, name="pB")
                terms = [
                    (ot_sb[NT - 1][:, ts(lt, P)], wo_bf[NT - 1][:, ts(nc2, 512)])
                ]
                pair_chain(
                    psA, psB, terms,
                    biasA=(ones_t[0:1, 0:P], bo_bf[0:1, ts(nc2, 512)]),
                )
                # ypart += B-half, then yt = A-half + ypart
                nc.vector.tensor_tensor(
                    ypart[lt][:, ts(nc2, 512)],
                    psB[:],
                    ypart[lt][:, ts(nc2, 512)],
                    ADD,
                )
                nc.vector.tensor_tensor(
                    yt[:, ts(nc2, 512)],
                    psA[:],
                    ypart[lt][:, ts(nc2, 512)],
                    ADD,
                )
            nc.sync.dma_start(t["y"][ts(lt, P), :], yt[:])

    prev = None
    for hp in range(NT):
        qt = emit_proj(hp, t["wqr"], xqT, "qt")
        kt_t = emit_proj(hp, t["wkr"], xkT, "kt")
        if prev is not None:
            emit_av(prev)
        if hp == NT - 1:
            outproj_batch1()
        prev = emit_scores(hp, qt, kt_t)
    emit_av(prev)
    outproj_batch2()


def build_nc(looped=False, reps=None, do_split=True):
    nc = bass.Bass("TRN2", debug=False, num_devices=N_CORES, num_swdge_queues=4)
    t = {}
    for name in ("xq", "xk", "xv"):
        t[name] = nc.dram_tensor(name, [L, D], BF16, kind="ExternalInput")
    for name in ("wv", "wo"):
        t[name] = nc.dram_tensor(name, [D, D], BF16, kind="ExternalInput")
    for name in ("wqr", "wkr"):
        t[name] = nc.dram_tensor(name, [NT * P, NT * P], BF16, kind="ExternalInput")
    for name in ("bkb", "bv", "bo"):
        t[name] = nc.dram_tensor(name, [D], BF16, kind="ExternalInput")
    t["ident"] = nc.dram_tensor("ident", [P, P], BF16, kind="ExternalInput")
    t["y"] = nc.dram_tensor("y", [L, D], BF16, kind="ExternalOutput")

    with tile.TileContext(nc) as tc:
        if reps is not None:
            loop_cm = tc.For_i(0, reps, 1)
        else:
            loop_cm = nullcontext()
        with loop_cm:
            with ExitStack() as ctx:
                _emit_body(nc, tc, ctx, t)

    if do_split:
        split_excess_waits(nc)
    return nc


# ---------------------------------------------------------------------------
# Runner: mirrors bass2jax.run_bass_via_pjrt's multi-core path, but keeps a
# reusable jitted callable (no donation) so repeated kernel() calls don't
# recompile.
def make_runner(nc, n_cores=N_CORES):
    import jax
    from jax.sharding import Mesh, NamedSharding, PartitionSpec
    from jax.experimental.shard_map import shard_map
    from concourse import bass2jax
    from concourse.bass2jax import _bass_exec_p, partition_id_tensor

    bass2jax.install_neuronx_cc_hook()

    partition_name = (
        nc.partition_id_tensor.name if nc.partition_id_tensor else None
    )
    in_names, out_names, out_avals, zero_outs = [], [], [], []
    for alloc in nc.m.functions[0].allocations:
        if not isinstance(alloc, mybir.MemoryLocationSet):
            continue
        name = alloc.memorylocations[0].name
        if alloc.kind == "ExternalInput":
            if name != partition_name:
                in_names.append(name)
        elif alloc.kind == "ExternalOutput":
            shape = tuple(alloc.tensor_shape)
            dtype = mybir.dt.np(alloc.dtype)
            out_names.append(name)
            out_avals.append(jax.core.ShapedArray(shape, dtype))
            zero_outs.append(np.zeros(shape, dtype))
    n_params = len(in_names)
    all_in_names = list(in_names) + list(out_names)
    if partition_name is not None:
        all_in_names.append(partition_name)

    def _body(*args):
        operands = list(args)
        if partition_name is not None:
            operands.append(partition_id_tensor())
        outs = _bass_exec_p.bind(
            *operands,
            out_avals=tuple(out_avals),
            in_names=tuple(all_in_names),
            out_names=tuple(out_names),
            lowering_input_output_aliases=(),
            sim_require_finite=True,
            sim_require_nnan=True,
            nc=nc,
        )
        return tuple(outs)

    devices = jax.devices()[:n_cores]
    mesh = Mesh(np.asarray(devices), ("core",))
    in_specs = (PartitionSpec("core"),) * (n_params + len(out_names))
    out_specs = (PartitionSpec("core"),) * len(out_names)
    fn = jax.jit(
        shard_map(
            _body, mesh=mesh, in_specs=in_specs, out_specs=out_specs,
            check_rep=False,
        ),
        keep_unused=True,
    )
    sharding = NamedSharding(mesh, PartitionSpec("core"))
    zeros_dev = [
        jax.device_put(
            np.zeros((n_cores * z.shape[0], *z.shape[1:]), z.dtype), sharding
        )
        for z in zero_outs
    ]

    def run(in_maps):
        per_core = [[np.asarray(m[n]) for n in in_names] for m in in_maps]
        concat_in = [
            np.concatenate([per_core[c][i] for c in range(n_cores)], axis=0)
            for i in range(n_params)
        ]
        args = [jax.device_put(a, sharding) for a in concat_in] + zeros_dev
        out = fn(*args)
        jax.block_until_ready(out)
        return [
            {
                n: np.asarray(out[i]).reshape(n_cores, *out_avals[i].shape)[c]
                for i, n in enumerate(out_names)
            }
            for c in range(n_cores)
        ]

    return run, fn, in_names, out_names, out_avals, sharding


_RUNNER = None


def _in_maps_from_inputs(inputs):
    import ml_dtypes

    bf = ml_dtypes.bfloat16
    ident = np.eye(P, dtype=bf)
    wq = np.asarray(inputs["Wq"], np.float32).astype(bf)
    wk = np.asarray(inputs["Wk"], np.float32).astype(bf)
    # [hp, p(c within ct), ct, dout] so each per-hp DMA reads contiguous
    # 2KB partition lines.
    wqr = np.ascontiguousarray(
        wq.reshape(NT, P, NT, P).transpose(2, 1, 0, 3)
    ).reshape(NT * P, NT * P)
    wkr = np.ascontiguousarray(
        wk.reshape(NT, P, NT, P).transpose(2, 1, 0, 3)
    ).reshape(NT * P, NT * P)
    wv = np.asarray(inputs["Wv"], np.float32).astype(bf)
    wo = np.asarray(inputs["Wo"], np.float32).astype(bf)
    bkb = np.asarray(inputs["bk"], np.float32).astype(bf)
    bv = np.asarray(inputs["bv"], np.float32).astype(bf)
    bo = np.asarray(inputs["bo"], np.float32).astype(bf)
    maps = []
    for b in range(N_CORES):
        m = {
            "xq": np.asarray(inputs["x_q"][b], np.float32).astype(bf),
            "xk": np.asarray(inputs["x_k"][b], np.float32).astype(bf),
            "xv": np.asarray(inputs["x_v"][b], np.float32).astype(bf),
            "wqr": wqr,
            "wkr": wkr,
            "wv": wv,
            "wo": wo,
            "bkb": bkb,
            "bv": bv,
            "bo": bo,
            "ident": ident,
        }
        maps.append(m)
    return maps


def kernel(**inputs) -> np.ndarray:
    global _RUNNER
    if _RUNNER is None:
        nc = build_nc()
        _RUNNER = make_runner(nc)[0]
    in_maps = _in_maps_from_inputs(inputs)
    results = _RUNNER(in_maps)
    out = np.stack([results[b]["y"] for b in range(N_CORES)], axis=0)
    return out.astype(np.float32)
